# revision 1
# baseline (speedup 1.0000x reference)
"""MoE (top-2 of 8 experts + dummy identity expert) on 8 NeuronCores.

Strategy (expert parallelism, per the sharding hint):
  Launch 1 (router, token-parallel): each core computes logits -> softmax
    -> top-2 gates for its 512-token shard, fully on device.
  Host all-to-all "dispatch": compact token indices per expert from the
    device-computed gates, gather+transpose token activations.
  Launch 2 (expert MLP, expert-parallel): core e holds expert e's weights
    and computes gate * (gelu(x @ w1) @ w2) for its (padded) token list.
  Host "combine": scatter-add per-expert outputs + dummy-expert term.

Matmuls run in bf16 with fp32 PSUM accumulation (configurable via DT).
"""

import math
import os
import sys

for _p in ("/opt/trn_rl_repo",):
    if _p not in sys.path:
        sys.path.insert(0, _p)

import numpy as np
import ml_dtypes

import concourse.bass as bass
import concourse.mybir as mybir
import concourse.tile as tile
from concourse.bass import ts
from concourse.bass_utils import run_bass_kernel_spmd
from concourse.vector_clock import ScopedClock

# ---------------------------------------------------------------------------
# This container's walrus accepts at most ONE sync-wait command per
# instruction. Tile can attach several (body instructions and the
# kernel-tail drain). Hoist excess waits onto same-engine NoOps inserted
# immediately before the offending instruction — semantically identical
# (waits are AND conditions evaluated in stream order).
# ---------------------------------------------------------------------------
_WAITS_PER_INST = 1
_legalize_counter = [0]


def _legalize_waits(nc):
    for f in nc.m.functions:
        for bb in f.blocks:
            insts = list(bb.instructions)
            out = []
            changed = False
            for inst in insts:
                si = inst.sync_info
                waits = list(si.on_wait) if si is not None and si.on_wait else []
                if len(waits) > _WAITS_PER_INST:
                    changed = True
                    for w in waits[:-_WAITS_PER_INST]:
                        _legalize_counter[0] += 1
                        out.append(
                            mybir.InstNoOp(
                                name=f"legwait-{_legalize_counter[0]}",
                                ins=[],
                                outs=[],
                                engine=inst.engine,
                                sync_info=mybir.SyncInfo(
                                    on_wait=[w], on_update=[]
                                ),
                            )
                        )
                    si.on_wait = waits[-_WAITS_PER_INST:]
                out.append(inst)
            if changed:
                bb.instructions = out
    return nc

# ---------------------------------------------------------------------------
# Problem constants (hardcoded per contract; inputs are fixed-shape).
# ---------------------------------------------------------------------------
N_CORES = 8
B, T, D, F, E = 2, 2048, 1024, 4096, 8
NTOK = B * T            # 4096 tokens
TPC = NTOK // N_CORES   # 512 tokens/core in the router launch
P = 128
KD = D // P             # 8 contraction chunks over D
KF = F // P             # 32 contraction chunks over F

F32 = mybir.dt.float32

# Matmul compute dtype. bfloat16: 1 PE cycle/row; float32: 4 cycles/row.
DT = mybir.dt.bfloat16
NP_DT = ml_dtypes.bfloat16

PROFILE = False          # set True (from test.py) to collect NTFF exec times
LAST_EXEC_NS = {}        # launch name -> exec_time_ns (filled when PROFILE)
LAST_TRACE_DIRS = {}


def _np_of(dt):
    return mybir.dt.np(dt)


# ---------------------------------------------------------------------------
# Launch 1: router. Per core: 512 tokens -> gates [512, 9].
# ---------------------------------------------------------------------------
def build_router():
    NE = E + 1
    TT = TPC // P  # 4 token tiles of 128
    nc = bass.Bass()
    xT = nc.declare_dram_parameter("xT", [KD, P, TPC], F32, isOutput=False)
    rw = nc.declare_dram_parameter("rw", [P, KD, NE], F32, isOutput=False)
    rb = nc.declare_dram_parameter("rb", [P, NE], F32, isOutput=False)
    id9 = nc.declare_dram_parameter("id9", [NE, NE], F32, isOutput=False)
    gates = nc.declare_dram_parameter("gates", [TPC, NE], F32, isOutput=True)

    with tile.TileContext(nc) as tc:
        with (
            tc.tile_pool(name="const", bufs=1) as cpool,
            tc.tile_pool(name="xp", bufs=4) as xpool,
            tc.tile_pool(name="work", bufs=2) as pool,
            tc.tile_pool(name="psum", bufs=1, space="PSUM") as pp,
            tc.tile_pool(name="psum2", bufs=1, space="PSUM") as pp2,
        ):
            rw_sb = cpool.tile([P, KD, NE], F32)
            nc.sync.dma_start(rw_sb[:], rw[:])
            rb_sb = cpool.tile([P, NE], F32)
            nc.gpsimd.dma_start(rb_sb[:], rb[:])
            id9_sb = cpool.tile([NE, NE], F32)
            nc.gpsimd.dma_start(id9_sb[:], id9[:])

            # logitsT [9, 512] with the 9-wide router weights stationary
            ps_lgT = pp2.tile([NE, TPC], F32)
            dma_engines = [nc.sync, nc.gpsimd, nc.scalar]
            for k in range(KD):
                xt_sb = xpool.tile([P, TPC], F32, tag="xt")
                dma_engines[k % 3].dma_start(xt_sb[:], xT[k])
                nc.tensor.matmul(
                    ps_lgT[:],
                    lhsT=rw_sb[:, k, :],
                    rhs=xt_sb[:],
                    start=(k == 0),
                    stop=(k == KD - 1),
                )
            lgT_sb = pool.tile([NE, TPC], F32, tag="lgT")
            nc.vector.tensor_copy(out=lgT_sb[:], in_=ps_lgT[:])

            # transpose back to token-major [128, 4, 9] via PE
            ps = pp.tile([P, TT, NE], F32)
            for tt in range(TT):
                nc.tensor.transpose(
                    ps[:, tt, :], lgT_sb[:, ts(tt, P)], id9_sb[:]
                )

            sh3 = [P, TT, NE]
            lg = pool.tile(sh3, F32, tag="lg")
            nc.vector.tensor_tensor(
                lg[:], ps[:], rb_sb[:, None, :].to_broadcast(sh3),
                mybir.AluOpType.add,
            )
            m1 = pool.tile([P, TT], F32, tag="m1")
            nc.vector.tensor_reduce(
                m1[:], lg[:], axis=mybir.AxisListType.X, op=mybir.AluOpType.max,
            )
            lgs = pool.tile(sh3, F32, tag="lgs")
            nc.vector.tensor_tensor(
                lgs[:], lg[:], m1[:, :, None].to_broadcast(sh3),
                mybir.AluOpType.subtract,
            )
            e_sb = pool.tile(sh3, F32, tag="e")
            nc.scalar.activation(
                e_sb[:], lgs[:], mybir.ActivationFunctionType.Exp,
            )
            s = pool.tile([P, TT], F32, tag="s")
            nc.vector.tensor_reduce(
                s[:], e_sb[:], axis=mybir.AxisListType.X, op=mybir.AluOpType.add,
            )
            mx = pool.tile([P, TT], F32, tag="mx")
            nc.vector.tensor_reduce(
                mx[:], e_sb[:], axis=mybir.AxisListType.X, op=mybir.AluOpType.max,
            )
            # knock out the top-1, take max again -> second-largest
            mlt = pool.tile(sh3, F32, tag="mlt")
            nc.vector.tensor_tensor(
                mlt[:], e_sb[:], mx[:, :, None].to_broadcast(sh3),
                mybir.AluOpType.is_lt,
            )
            emask = pool.tile(sh3, F32, tag="emask")
            nc.vector.tensor_mul(out=emask[:], in0=mlt[:], in1=e_sb[:])
            m2 = pool.tile([P, TT], F32, tag="m2")
            nc.vector.tensor_reduce(
                m2[:], emask[:], axis=mybir.AxisListType.X, op=mybir.AluOpType.max,
            )
            gmask = pool.tile(sh3, F32, tag="gmask")
            nc.vector.tensor_tensor(
                gmask[:], e_sb[:], m2[:, :, None].to_broadcast(sh3),
                mybir.AluOpType.is_ge,
            )
            gsel = pool.tile(sh3, F32, tag="gsel")
            nc.vector.tensor_mul(out=gsel[:], in0=gmask[:], in1=e_sb[:])
            rs = pool.tile([P, TT], F32, tag="rs")
            nc.vector.reciprocal(rs[:], s[:])
            gfin = pool.tile(sh3, F32, tag="gfin")
            nc.vector.tensor_tensor(
                gfin[:], gsel[:], rs[:, :, None].to_broadcast(sh3),
                mybir.AluOpType.mult,
            )
            nc.sync.dma_start(
                gates.rearrange("(tt p) e -> p tt e", p=P), gfin[:]
            )
    return _legalize_waits(nc)


# ---------------------------------------------------------------------------
# Launch 2: expert MLP. Per core: capacity-C tokens through one expert.
#   yT[d, c] = gate[c] * (gelu(x @ w1) @ w2)[c, d]   (transposed output)
# ---------------------------------------------------------------------------
def build_mlp(C):
    assert C % 512 == 0
    NT = C // 512
    nc = bass.Bass()
    w1 = nc.declare_dram_parameter("w1", [KF, P, KD, P], DT, isOutput=False)
    w2 = nc.declare_dram_parameter("w2", [KD, P, KF, P], DT, isOutput=False)
    xT = nc.declare_dram_parameter("xT", [NT, P, KD, 512], DT, isOutput=False)
    grep = nc.declare_dram_parameter("grep", [P, C], F32, isOutput=False)
    yT = nc.declare_dram_parameter("yT", [D, C], F32, isOutput=True)

    with tile.TileContext(nc) as tc:
        with (
            tc.tile_pool(name="const", bufs=1) as cpool,
            tc.tile_pool(name="w1p", bufs=4) as w1pool,
            tc.tile_pool(name="w2p", bufs=2) as w2pool,
            tc.tile_pool(name="yp", bufs=3) as ypool,
            tc.tile_pool(name="psh", bufs=2, space="PSUM") as pph,
            tc.tile_pool(name="psy", bufs=2, space="PSUM") as ppy,
        ):
            xt_tiles = []
            for t in range(NT):
                xt_sb = cpool.tile([P, KD, 512], DT, tag=f"xt{t}")
                nc.sync.dma_start(xt_sb[:], xT[t])
                xt_tiles.append(xt_sb)
            hT_sb = cpool.tile([P, KF, C], DT)

            # phase 1: hT[f, c] = gelu(sum_k w1[k, f] * x[k, c])
            for f in range(KF):
                w1_sb = w1pool.tile([P, KD, P], DT, tag="w1t")
                (nc.sync if f < 2 else nc.gpsimd).dma_start(w1_sb[:], w1[f])
                for t in range(NT):
                    ps = pph.tile([P, 512], F32)
                    for k in range(KD):
                        nc.tensor.matmul(
                            ps[:],
                            lhsT=w1_sb[:, k, :],
                            rhs=xt_tiles[t][:, k, :],
                            start=(k == 0),
                            stop=(k == KD - 1),
                        )
                    nc.scalar.activation(
                        hT_sb[:, f, ts(t, 512)], ps[:],
                        mybir.ActivationFunctionType.Gelu,
                    )

            # phase 2: yT[d, c] = gate[c] * sum_k w2[k, d] * hT[k, c]
            grep_sb = cpool.tile([P, C], F32)
            nc.gpsimd.dma_start(grep_sb[:], grep[:])
            for d in range(KD):
                w2_sb = w2pool.tile([P, KF, P], DT)
                nc.gpsimd.dma_start(w2_sb[:], w2[d])
                for t in range(NT):
                    ps = ppy.tile([P, 512], F32)
                    for k in range(KF):
                        nc.tensor.matmul(
                            ps[:],
                            lhsT=w2_sb[:, k, :],
                            rhs=hT_sb[:, k, ts(t, 512)],
                            start=(k == 0),
                            stop=(k == KF - 1),
                        )
                    y_sb = ypool.tile([P, 512], F32)
                    nc.vector.tensor_mul(
                        out=y_sb[:], in0=ps[:], in1=grep_sb[:, ts(t, 512)]
                    )
                    nc.scalar.dma_start(yT[ts(d, P), ts(t, 512)], y_sb[:])
    return _legalize_waits(nc)


_BUILT = {}


def _get_router():
    if "router" not in _BUILT:
        _BUILT["router"] = build_router()
    return _BUILT["router"]


def _get_mlp(C):
    key = ("mlp", C)
    if key not in _BUILT:
        _BUILT[key] = build_mlp(C)
    return _BUILT[key]


def _run(name, nc, in_maps):
    kw = {}
    if PROFILE:
        kw["trace"] = True
    res = run_bass_kernel_spmd(nc, in_maps, core_ids=list(range(N_CORES)), **kw)
    if PROFILE:
        LAST_EXEC_NS[name] = res.exec_time_ns
        LAST_TRACE_DIRS[name] = getattr(res, "profile_json", None)
    return res.results


# ---------------------------------------------------------------------------
# host-side packing helpers
# ---------------------------------------------------------------------------
def _part3(a, np_dt):
    """[K*P, N] -> [P, K, N] with partition dim first (contiguous)."""
    kp, n = a.shape
    k = kp // P
    return np.ascontiguousarray(
        a.reshape(k, P, n).transpose(1, 0, 2).astype(np_dt, copy=False)
    )


def kernel(x, router_w, router_b, w1, w2):
    x = np.asarray(x, dtype=np.float32)
    router_w = np.asarray(router_w, dtype=np.float32)
    router_b = np.asarray(router_b, dtype=np.float32)
    w1 = np.asarray(w1, dtype=np.float32)
    w2 = np.asarray(w2, dtype=np.float32)

    xf = x.reshape(NTOK, D)
    np_dt = _np_of(DT)

    # ---- launch 1: router -------------------------------------------------
    rw_h = _part3(router_w, np.float32)                    # [128, 8, 9]
    rb_h = np.ascontiguousarray(np.broadcast_to(router_b, (P, E + 1)))
    id9_h = np.eye(E + 1, dtype=np.float32)
    in_maps = []
    for c in range(N_CORES):
        xs = xf[c * TPC:(c + 1) * TPC]                     # [512, 1024]
        xT_h = np.ascontiguousarray(xs.T).reshape(KD, P, TPC)  # [8, 128, 512]
        in_maps.append({"xT": xT_h, "rw": rw_h, "rb": rb_h, "id9": id9_h})
    results = _run("router", _get_router(), in_maps)
    gates = np.concatenate(
        [np.asarray(r["gates"], dtype=np.float32) for r in results], axis=0
    )                                                      # [4096, 9]

    # ---- host all-to-all dispatch ----------------------------------------
    idx = [np.nonzero(gates[:, e] > 0)[0] for e in range(E)]
    maxc = max(len(i) for i in idx)
    C = max(512, ((maxc + 511) // 512) * 512)

    nc_mlp = _get_mlp(C)
    in_maps = []
    for e in range(E):
        ide = idx[e]
        ne = len(ide)
        xg = np.zeros((C, D), dtype=np.float32)
        xg[:ne] = xf[ide]
        g = np.zeros((C,), dtype=np.float32)
        g[:ne] = gates[ide, e]
        w2_blocks = np.stack(
            [
                w2[e][:, d * P:(d + 1) * P]
                .reshape(KF, P, P).transpose(1, 0, 2)
                for d in range(KD)
            ]
        ).astype(np_dt, copy=False)                        # [8, 128, 32, 128]
        w1_blocks = np.stack(
            [
                w1[e][:, f * P:(f + 1) * P]
                .reshape(KD, P, P).transpose(1, 0, 2)
                for f in range(KF)
            ]
        ).astype(np_dt, copy=False)                        # [32, 128, 8, 128]
        xT_b = _part3(np.ascontiguousarray(xg.T), np_dt)   # [128, 8, C]
        xT_b = np.ascontiguousarray(
            xT_b.reshape(P, KD, C // 512, 512).transpose(2, 0, 1, 3)
        )                                                  # [NT, 128, 8, 512]
        in_maps.append({
            "w1": np.ascontiguousarray(w1_blocks),
            "w2": np.ascontiguousarray(w2_blocks),
            "xT": xT_b,
            "grep": np.ascontiguousarray(np.broadcast_to(g, (P, C))),
        })

    # ---- launch 2: expert MLP --------------------------------------------
    results = _run("mlp", nc_mlp, in_maps)

    # ---- host combine -----------------------------------------------------
    out = gates[:, E:E + 1] * xf                           # dummy identity expert
    for e in range(E):
        ne = len(idx[e])
        if ne == 0:
            continue
        yT = np.asarray(results[e]["yT"], dtype=np.float32)    # [1024, C]
        out[idx[e]] += yT.T[:ne]
    return out.reshape(B, T, D).astype(np.float32)

